# revision 1
# baseline (speedup 1.0000x reference)
"""Multi-head attention (B=2, N=2048, D=1024, H=16, HD=64) on 8 trn2 NeuronCores.

Sharding: data-parallel over batch (2) x tensor-parallel over head groups (4).
Core c handles batch b=c//4, heads 4*(c%4)..4*(c%4)+3. Each core computes
Q/K/V projections for its head slice, attention, and a partial output
projection (its heads' rows of Wo); the host sums the 4 partials per batch
and adds bo.

Device layout strategy: everything lives feature-on-partitions ("transposed")
so no on-device transposes are needed:
  - host passes X[b].T; Q^T/K^T computed as (W^T X^T) with W as stationary.
  - V computed in native [token, d] layout (X^T tiles as stationary).
  - scores computed as S^T[j, i] (key j on partitions) so the mask bias is a
    per-partition scalar and softmax normalization is deferred:
    E^T = exp(S/8 + maskbias) via one ScalarE activation (PSUM->SBUF).
  - ctx^T[d, i] = sum_j V_aug[j, d] E^T[j, i]; V_aug has a ones column so the
    softmax denominator rides along as ctx row 64.
  - normalization multiplies ctx^T by 1/denom broadcast via a tiny PE matmul.
  - out^T = Wo^T ctx^T accumulated over head pairs; host transposes back.
"""

import sys

if "/opt/trn_rl_repo" not in sys.path:
    sys.path.insert(0, "/opt/trn_rl_repo")

import numpy as np

import concourse.bacc as bacc
import concourse.mybir as mybir
import concourse.tile as tile

B, N, D = 2, 2048, 1024
H, HD = 16, 64
HG = 4  # head groups (tensor parallel)
HPG = H // HG  # heads per group = 4
DG = HPG * HD  # feature slice per group = 256

F32 = mybir.dt.float32
# Matmul datapath dtype: float32r is the fast (1 cycle/row at N>=256) fp32
# matmul mode; tiles and DRAM tensors feeding matmuls must be typed fp32r.
MMT = mybir.dt.float32r


def _mm_ap(ap):
    return ap


def build_program(loop_iters: int = 1):
    nc = bacc.Bacc("TRN2", target_bir_lowering=False)

    xt = nc.dram_tensor("xt", [D, N], MMT, kind="ExternalInput")
    wq = nc.dram_tensor("wq", [128, 8, DG], MMT, kind="ExternalInput")
    wk = nc.dram_tensor("wk", [128, 8, DG], MMT, kind="ExternalInput")
    wv = nc.dram_tensor("wv", [128, 8, DG], MMT, kind="ExternalInput")
    bq = nc.dram_tensor("bq", [128, 2], F32, kind="ExternalInput")
    bk = nc.dram_tensor("bk", [128, 2], F32, kind="ExternalInput")
    bvr = nc.dram_tensor("bvr", [1, DG], MMT, kind="ExternalInput")
    wo = nc.dram_tensor("wo", [128, 2, D], MMT, kind="ExternalInput")
    mb = nc.dram_tensor("mb", [128, 16], F32, kind="ExternalInput")
    onesin = nc.dram_tensor("onesin", [128, 128], MMT, kind="ExternalInput")
    outp = nc.dram_tensor("outp", [D, N], F32, kind="ExternalOutput")

    with tile.TileContext(nc) as tc, nc.allow_low_precision(
        reason="fp32r matmul datapath; accumulation stays fp32 in PSUM"
    ):
        import contextlib

        ctx = contextlib.ExitStack()
        with ctx:
            const = ctx.enter_context(tc.tile_pool(name="const", bufs=1))
            big = ctx.enter_context(tc.tile_pool(name="big", bufs=5))
            xtcp = ctx.enter_context(tc.tile_pool(name="xtcp", bufs=4))
            qk = ctx.enter_context(tc.tile_pool(name="qk", bufs=1))
            epool = ctx.enter_context(tc.tile_pool(name="epool", bufs=5))
            rpool = ctx.enter_context(tc.tile_pool(name="rpool", bufs=2))
            psum_b = ctx.enter_context(
                tc.tile_pool(name="psum_b", bufs=2, space="PSUM")
            )
            psum_c = ctx.enter_context(
                tc.tile_pool(name="psum_c", bufs=2, space="PSUM")
            )

            loop_cm = (
                tc.For_i(0, loop_iters, 1)
                if loop_iters > 1
                else contextlib.nullcontext()
            )
            with loop_cm:
                # ---- phase 1 loads first: wq + X chunk 0 gate the first
                # matmul chain, so issue them before the small const DMAs.
                wq_sb = big.tile([128, 8, DG], MMT, tag="big")
                nc.sync.dma_start(out=wq_sb[:, :, 0:128], in_=wq[:, :, 0:128])
                nc.sync.dma_start(out=wq_sb[:, :, 128:DG], in_=wq[:, :, 128:DG])
                wk_sb = big.tile([128, 8, DG], MMT, tag="big")
                nc.scalar.dma_start(out=wk_sb, in_=wk[:, :, :])
                wv_sb = big.tile([128, 8, DG], MMT, tag="big")
                nc.scalar.dma_start(out=wv_sb, in_=wv[:, :, :])

                # ---- constants ----
                ones = const.tile([128, 128], MMT, tag="ones")
                nc.sync.dma_start(out=ones, in_=onesin[:, :])
                bq_sb = const.tile([128, 2], F32, tag="bq")
                nc.sync.dma_start(out=bq_sb, in_=bq[:, :])
                bk_sb = const.tile([128, 2], F32, tag="bk")
                nc.sync.dma_start(out=bk_sb, in_=bk[:, :])
                bvr_sb = const.tile([1, DG], MMT, tag="bvr")
                nc.sync.dma_start(out=bvr_sb, in_=bvr[:, :])
                mb_sb = const.tile([128, 16], F32, tag="mb")
                nc.sync.dma_start(out=mb_sb, in_=mb[:, :])
                wo_sb = const.tile([128, 2, D], MMT, tag="wo")

                # bv broadcast to all 128 partitions via PE
                bv_ps = psum_b.tile([128, DG], F32, tag="bank")
                nc.tensor.matmul(
                    bv_ps, _mm_ap(ones[0:1, 0:128]), _mm_ap(bvr_sb[0:1, :]),
                    start=True, stop=True,
                )
                bv_bc = const.tile([128, DG], F32, tag="bvbc")
                nc.vector.tensor_copy(bv_bc, bv_ps)
                # X^T in 4 column-chunk tiles [128, kt=8, 512]; one DMA each,
                # chunk-major so early Q/K/V chains start before the full X
                # load completes. xtc[c][kt] view = features kt*128.., tokens
                # c*512..
                xtc_t = []
                for c in range(4):
                    t = xtcp.tile([128, 8, 512], MMT, tag="xtc", name="xtc")
                    eng = nc.sync if c < 2 else nc.scalar
                    nparts = 4 if c == 0 else 2
                    for h in range(nparts):
                        kpp = 8 // nparts
                        eng.dma_start(
                            out=t[:, h * kpp : (h + 1) * kpp, :],
                            in_=xt[
                                h * kpp * 128 : (h + 1) * kpp * 128,
                                c * 512 : (c + 1) * 512,
                            ].rearrange("(kt p) col -> p kt col", p=128),
                        )
                    xtc_t.append(t)
                xtc = [[xtc_t[c][:, kt, :] for kt in range(8)] for c in range(4)]

                qt_sb = [qk.tile([128, N], MMT, tag=f"qt{m}", name=f"qt{m}") for m in range(2)]
                kt_sb = [qk.tile([128, N], MMT, tag=f"kt{m}", name=f"kt{m}") for m in range(2)]
                # V with ones column appended per head: [128, jt, head, 65]
                v_sb = qk.tile([128, 16, HPG, HD + 1], MMT, tag="v")
                nc.scalar.dma_start(
                    out=v_sb[:, :, :, HD : HD + 1], in_=onesin[:, 0:64]
                )

                def qk_chain(proj, mt, nt):
                    w_sb, bias_sb, dst = (
                        (wq_sb, bq_sb, qt_sb) if proj == 0 else (wk_sb, bk_sb, kt_sb)
                    )
                    ps = psum_b.tile([128, 512], F32, tag="bank", name="qkps")
                    for kt in range(8):
                        nc.tensor.matmul(
                            ps,
                            _mm_ap(w_sb[:, kt, mt * 128 : (mt + 1) * 128]),
                            _mm_ap(xtc[nt][kt]),
                            start=(kt == 0),
                            stop=(kt == 7),
                        )
                    nc.vector.tensor_scalar_add(
                        dst[mt][:, nt * 512 : (nt + 1) * 512],
                        ps,
                        bias_sb[:, mt : mt + 1],
                    )

                def v_chain(mt):
                    ps = psum_b.tile([128, DG], F32, tag="bank", name="vps")
                    for kt in range(8):
                        nc.tensor.matmul(
                            ps,
                            _mm_ap(
                                xtc[mt // 4][kt][
                                    :, (mt % 4) * 128 : (mt % 4 + 1) * 128
                                ]
                            ),
                            _mm_ap(wv_sb[:, kt, :]),
                            start=(kt == 0),
                            stop=(kt == 7),
                        )
                    nc.vector.tensor_tensor(
                        out=v_sb[:, mt, :, 0:HD],
                        in0=ps.rearrange("p (h d) -> p h d", h=HPG),
                        in1=bv_bc.rearrange("p (h d) -> p h d", h=HPG),
                        op=mybir.AluOpType.add,
                    )

                # chains needed before the unit stream starts
                for fn in (
                    lambda: qk_chain(0, 0, 0),
                    lambda: qk_chain(1, 0, 0),
                    lambda: qk_chain(0, 0, 1),
                    lambda: qk_chain(1, 0, 1),
                    lambda: v_chain(0),
                ):
                    fn()

                # remaining chains, spread through the unit stream (key =
                # iteration index at whose END the chain is emitted; each must
                # precede its first consumer unit)
                inserts = {}
                for i in range(1, 16):
                    inserts.setdefault(i, []).append(lambda m=i: v_chain(m))
                inserts.setdefault(6, []).append(lambda: qk_chain(1, 0, 2))
                inserts.setdefault(8, []).append(
                    lambda: nc.sync.dma_start(out=wo_sb, in_=wo[:, :, :])
                )
                inserts.setdefault(10, []).append(lambda: qk_chain(1, 0, 3))
                inserts.setdefault(11, []).append(lambda: qk_chain(0, 1, 0))
                inserts.setdefault(12, []).append(lambda: qk_chain(1, 1, 0))
                inserts.setdefault(13, []).append(lambda: qk_chain(0, 1, 1))
                inserts.setdefault(17, []).append(lambda: qk_chain(1, 1, 1))
                inserts.setdefault(21, []).append(lambda: qk_chain(1, 1, 2))
                inserts.setdefault(25, []).append(lambda: qk_chain(1, 1, 3))
                inserts.setdefault(28, []).append(lambda: qk_chain(0, 0, 2))
                inserts.setdefault(30, []).append(lambda: qk_chain(0, 0, 3))
                inserts.setdefault(44, []).append(lambda: qk_chain(0, 1, 2))
                inserts.setdefault(46, []).append(lambda: qk_chain(0, 1, 3))


                # ---- phase 2: attention, software-pipelined emission ----
                ctxn = [
                    qk.tile([128, N], MMT, tag=f"ctxn{m}", name=f"ctxn{m}")
                    for m in range(2)
                ]

                blocks = [(ih, hp) for ih in range(2) for hp in range(2)]
                units = [
                    (b_idx, ih, hp, jt)
                    for b_idx, (ih, hp) in enumerate(blocks)
                    for jt in range(16)
                ]
                ctx_ps_of = {}
                unit_e = {}

                def emit_s_exp(u):
                    b_idx, ih, hp, jt = u
                    e_sb = [
                        epool.tile([128, 1024], MMT, tag="e", name="esb")
                        for _ in range(2)
                    ]
                    s_ps2 = [
                        psum_b.tile([128, 1024], F32, tag="bank", name="sps")
                        for _ in range(2)
                    ]
                    for h2 in range(2):
                        for it in range(2):
                            nc.tensor.matmul(
                                s_ps2[h2][:, it * 512 : (it + 1) * 512],
                                _mm_ap(
                                    kt_sb[hp][
                                        h2 * 64 : (h2 + 1) * 64,
                                        jt * 128 : (jt + 1) * 128,
                                    ]
                                ),
                                _mm_ap(
                                    qt_sb[hp][
                                        h2 * 64 : (h2 + 1) * 64,
                                        ih * 1024 + it * 512 : ih * 1024
                                        + (it + 1) * 512,
                                    ]
                                ),
                                start=True,
                                stop=True,
                            )
                    for h2 in range(2):
                        nc.scalar.activation(
                            out=e_sb[h2],
                            in_=s_ps2[h2],
                            func=mybir.ActivationFunctionType.Exp,
                            bias=mb_sb[:, jt : jt + 1],
                            scale=0.125,
                        )
                    unit_e[u] = e_sb

                def emit_ctx(u):
                    b_idx, ih, hp, jt = u
                    if b_idx not in ctx_ps_of:
                        ctx_ps_of[b_idx] = [
                            psum_c.tile([HD + 1, 1024], F32, tag="ctx", name="ctxps")
                            for _ in range(2)
                        ]
                    ctx_ps = ctx_ps_of[b_idx]
                    e_sb = unit_e.pop(u)
                    for h2 in range(2):
                        for it in range(2):
                            nc.tensor.matmul(
                                ctx_ps[h2][:, it * 512 : (it + 1) * 512],
                                _mm_ap(v_sb[:, jt, 2 * hp + h2, :]),
                                _mm_ap(e_sb[h2][:, it * 512 : (it + 1) * 512]),
                                start=(jt == 0),
                                stop=(jt == 15),
                                skip_group_check=True,
                            )

                def emit_norm(b_idx):
                    ih, hp = blocks[b_idx]
                    ctx_ps = ctx_ps_of[b_idx]
                    for h2 in (1, 0):
                        r_sb = rpool.tile([65, 1024], MMT, tag="r", name="rsb")
                        nc.vector.reciprocal(
                            out=r_sb[64:65, :], in_=ctx_ps[h2][64:65, :]
                        )
                        for it in range(2):
                            rp = psum_b.tile([64, 512], F32, tag="bank", name="rp")
                            nc.tensor.matmul(
                                rp,
                                _mm_ap(ones[64:65, 0:64]),
                                _mm_ap(r_sb[64:65, it * 512 : (it + 1) * 512]),
                                start=True,
                                stop=True,
                                tile_position=(64, 0),
                            )
                            nc.scalar.copy(
                                r_sb[0:64, it * 512 : (it + 1) * 512], rp
                            )
                        if h2 == 0:
                            nc.vector.tensor_tensor(
                                out=ctxn[hp][0:64, ih * 1024 : (ih + 1) * 1024],
                                in0=ctx_ps[0][0:64, :],
                                in1=r_sb[0:64, :],
                                op=mybir.AluOpType.mult,
                            )
                        else:
                            tmp = big.tile([64, 1024], MMT, tag="big", name="tmp")
                            nc.vector.tensor_tensor(
                                out=tmp,
                                in0=ctx_ps[1][0:64, :],
                                in1=r_sb[0:64, :],
                                op=mybir.AluOpType.mult,
                            )
                            # partition shift 0-63 -> 64-127 via SBUF->SBUF DMA
                            nc.sync.dma_start(
                                out=ctxn[hp][64:128, ih * 1024 : (ih + 1) * 1024],
                                in_=tmp,
                            )

                def emit_outproj(ih, mo_list=None, copy_eng=None):
                    for mo in (range(8) if mo_list is None else mo_list):
                        ps = psum_b.tile([128, 1024], F32, tag="bank", name="ops")
                        for nt2 in range(2):
                            nt = 2 * ih + nt2
                            for kt in range(2):
                                nc.tensor.matmul(
                                    ps[:, nt2 * 512 : (nt2 + 1) * 512],
                                    _mm_ap(wo_sb[:, kt, mo * 128 : (mo + 1) * 128]),
                                    _mm_ap(ctxn[kt][:, nt * 512 : (nt + 1) * 512]),
                                    start=(kt == 0),
                                    stop=(kt == 1),
                                )
                        ob = big.tile([128, 1024], F32, tag="big", name="ob")
                        (copy_eng or nc.vector.tensor_copy)(ob, ps)
                        nc.sync.dma_start(
                            out=outp[
                                mo * 128 : (mo + 1) * 128,
                                ih * 1024 : (ih + 1) * 1024,
                            ],
                            in_=ob,
                        )

                extras = {}
                for b_idx, (ih, hp) in enumerate(blocks):
                    last = 16 * (b_idx + 1) - 1
                    extras.setdefault(last + 1, []).append(
                        lambda b=b_idx: emit_norm(b)
                    )
                    if hp == 1:
                        for j, mo in enumerate(range(8)):
                            # tail (ih==1): both ScalarE and DVE are idle, so
                            # alternate the psum->sbuf copies across them to
                            # halve the final drain
                            ce = (
                                (nc.scalar.copy if mo % 2 == 0 else None)
                                if ih == 1
                                else None
                            )
                            extras.setdefault(last + 3 + j, []).append(
                                lambda i=ih, m=mo, c=ce: emit_outproj(i, [m], c)
                            )

                trailing = []
                for i, u in enumerate(units):
                    emit_s_exp(u)
                    if i > 0:
                        emit_ctx(units[i - 1])
                    for fn in inserts.get(i, []):
                        fn()
                    for fn in extras.get(i, []):
                        if i == len(units) - 1:
                            trailing.append(fn)
                        else:
                            fn()
                emit_ctx(units[-1])
                for i in sorted(extras):
                    if i >= len(units):
                        trailing.extend(extras[i])
                for fn in trailing:
                    fn()

    nc.finalize()
    return nc


_NC_CACHE = None


def _get_program():
    global _NC_CACHE
    if _NC_CACHE is None:
        _NC_CACHE = build_program()
    return _NC_CACHE


def make_in_maps(X, mask, Wq, bq, Wk, bk, Wv, bv, Wo, bo):
    X = np.asarray(X, dtype=np.float32)
    mask = np.asarray(mask, dtype=np.float32)
    in_maps = []
    xts = [np.ascontiguousarray(X[b].T) for b in range(B)]
    mbs = [
        np.ascontiguousarray((-1e6 * (1.0 - mask[b])).reshape(16, 128).T)
        for b in range(B)
    ]
    for c in range(8):
        b, g = c // HG, c % HG
        sl = slice(g * DG, (g + 1) * DG)
        wq_s = np.ascontiguousarray(
            np.asarray(Wq[:, sl]).reshape(8, 128, DG).transpose(1, 0, 2)
        )
        wk_s = np.ascontiguousarray(
            np.asarray(Wk[:, sl]).reshape(8, 128, DG).transpose(1, 0, 2)
        )
        wv_s = np.ascontiguousarray(
            np.asarray(Wv[:, sl]).reshape(8, 128, DG).transpose(1, 0, 2)
        )
        bq_s = np.ascontiguousarray(np.asarray(bq[sl]).reshape(2, 128).T)
        bk_s = np.ascontiguousarray(np.asarray(bk[sl]).reshape(2, 128).T)
        bv_s = np.ascontiguousarray(np.asarray(bv[sl]).reshape(1, DG))
        # Wo rows for this group, pair-packed: [64*h2+p, kt, o] = Wo[g*256+(2kt+h2)*64+p, o]
        wo_s = np.ascontiguousarray(
            np.asarray(Wo[sl, :]).reshape(2, 2, 64, D).transpose(1, 2, 0, 3)
            .reshape(128, 2, D)
        )
        in_maps.append(
            {
                "xt": xts[b],
                "onesin": np.ones((128, 128), dtype=np.float32),
                "wq": wq_s.astype(np.float32),
                "wk": wk_s.astype(np.float32),
                "wv": wv_s.astype(np.float32),
                "bq": bq_s.astype(np.float32),
                "bk": bk_s.astype(np.float32),
                "bvr": bv_s.astype(np.float32),
                "wo": wo_s.astype(np.float32),
                "mb": mbs[b].astype(np.float32),
            }
        )
    return in_maps


def gather_output(results, bo):
    out = np.zeros((B, N, D), dtype=np.float32)
    for c in range(8):
        out[c // HG] += results[c]["outp"].T
    out += np.asarray(bo, dtype=np.float32)
    return out


def kernel(**inputs):
    from concourse import bass_utils

    nc = _get_program()
    in_maps = make_in_maps(**inputs)
    res = bass_utils.run_bass_kernel_spmd(nc, in_maps, core_ids=list(range(8)))
    return gather_output(res.results, inputs["bo"])



# revision 27
# speedup vs baseline: 11.0638x; 11.0638x over previous
"""Multi-head attention (B=2, N=2048, D=1024, H=16, HD=64) on 8 trn2 NeuronCores.

Sharding: data-parallel over batch (2) x tensor-parallel over head groups (4).
Core c handles batch b=c//4, heads 4*(c%4)..4*(c%4)+3. Each core computes
Q/K/V projections for its head slice, attention, and a partial output
projection (its heads' rows of Wo); the host sums the 4 partials per batch
and adds bo.

Device layout strategy: everything lives feature-on-partitions ("transposed")
so no on-device transposes are needed:
  - host passes X[b].T; Q^T/K^T computed as (W^T X^T) with W as stationary.
  - V computed in native [token, d] layout (X^T tiles as stationary).
  - scores computed as S^T[j, i] (key j on partitions) so the mask bias is a
    per-partition scalar and softmax normalization is deferred:
    E^T = exp(S/8 + maskbias) via one ScalarE activation (PSUM->SBUF).
  - ctx^T[d, i] = sum_j V_aug[j, d] E^T[j, i]; V_aug has a ones column so the
    softmax denominator rides along as ctx row 64.
  - normalization multiplies ctx^T by 1/denom broadcast via a tiny PE matmul.
  - out^T = Wo^T ctx^T accumulated over head pairs; host transposes back.

Performance notes vs the fp32r baseline:
  - whole matmul datapath in bf16 (X, Wq/k/v/o, Q^T/K^T, V, E, ctxn): same
    1 cycle/row PE rate but half the SBUF/DMA traffic, and FWL weight loads.
  - score matmuls (K=64) are emitted pairwise with explicit tile_position
    (0,0)/(64,0) so the two heads' matmuls run concurrently in the PE array.
  - ScalarE runs nothing but the 128 exp activations; all DMA triggers sit
    on sync/pool/vector queues and norm/tail copies on DVE.
  - insert schedule spreads projection/outproj chains into the ACT-bound
    stretch so the PE never starves the exp stream.
"""

import sys

if "/opt/trn_rl_repo" not in sys.path:
    sys.path.insert(0, "/opt/trn_rl_repo")

import numpy as np

import concourse.bacc as bacc
import concourse.mybir as mybir
import concourse.tile as tile

B, N, D = 2, 2048, 1024
H, HD = 16, 64
HG = 4  # head groups (tensor parallel)
HPG = H // HG  # heads per group = 4
DG = HPG * HD  # feature slice per group = 256

F32 = mybir.dt.float32
MMT = mybir.dt.bfloat16  # matmul datapath dtype

# feature flags (for HW-vs-sim bisection)
USE_PBCAST = False     # gpsimd.partition_broadcast in emit_norm
USE_TILEPOS = True     # explicit tile_position on score matmuls
USE_GPSIMD_DMA = True  # DMA triggers on the Pool (swdge) queue


def build_program(loop_iters: int = 1):
    nc = bacc.Bacc("TRN2", target_bir_lowering=False)

    xt = nc.dram_tensor("xt", [D, N], MMT, kind="ExternalInput")
    wq = nc.dram_tensor("wq", [128, 8, DG], MMT, kind="ExternalInput")
    wk = nc.dram_tensor("wk", [128, 8, DG], MMT, kind="ExternalInput")
    wv = nc.dram_tensor("wv", [128, 8, DG], MMT, kind="ExternalInput")
    bq = nc.dram_tensor("bq", [128, 2], F32, kind="ExternalInput")
    bk = nc.dram_tensor("bk", [128, 2], F32, kind="ExternalInput")
    bvr = nc.dram_tensor("bvr", [1, DG], MMT, kind="ExternalInput")
    wo = nc.dram_tensor("wo", [128, 2, D], MMT, kind="ExternalInput")
    mb = nc.dram_tensor("mb", [128, 16], F32, kind="ExternalInput")
    onesin = nc.dram_tensor("onesin", [128, 128], MMT, kind="ExternalInput")
    outp = nc.dram_tensor("outp", [D, N], F32, kind="ExternalOutput")

    with tile.TileContext(nc) as tc, nc.allow_low_precision(
        reason="bf16 matmul datapath; accumulation stays fp32 in PSUM"
    ):
        import contextlib

        ctx = contextlib.ExitStack()
        with ctx:
            const = ctx.enter_context(tc.tile_pool(name="const", bufs=1))
            big = ctx.enter_context(tc.tile_pool(name="big", bufs=5))
            xtcp = ctx.enter_context(tc.tile_pool(name="xtcp", bufs=4))
            qk = ctx.enter_context(tc.tile_pool(name="qk", bufs=1))
            epool = ctx.enter_context(tc.tile_pool(name="epool", bufs=5))
            rpool = ctx.enter_context(tc.tile_pool(name="rpool", bufs=2))
            psum_b = ctx.enter_context(
                tc.tile_pool(name="psum_b", bufs=2, space="PSUM")
            )
            psum_c = ctx.enter_context(
                tc.tile_pool(name="psum_c", bufs=2, space="PSUM")
            )

            loop_cm = (
                tc.For_i(0, loop_iters, 1)
                if loop_iters > 1
                else contextlib.nullcontext()
            )
            with loop_cm:
                # ---- phase 1 loads: wq halves + X chunk 0 gate the first
                # matmul chain. Spread triggers over sync/scalar/gpsimd so the
                # gating transfers hit separate DMA rings immediately (the
                # scalar queue is free until the first exp ~6us in).
                xtc_t = [
                    xtcp.tile([128, 8, 512], MMT, tag="xtc", name="xtc")
                    for _ in range(4)
                ]

                def g(eng):
                    if eng is nc.gpsimd and not USE_GPSIMD_DMA:
                        return nc.scalar
                    return eng

                def xtc_dma(eng, c, k0, k1):
                    eng = g(eng)
                    eng.dma_start(
                        out=xtc_t[c][:, k0:k1, :],
                        in_=xt[
                            k0 * 128 : k1 * 128, c * 512 : (c + 1) * 512
                        ].rearrange("(kt p) col -> p kt col", p=128),
                    )

                wq_sb = big.tile([128, 8, DG], MMT, tag="big")
                wk_sb = big.tile([128, 8, DG], MMT, tag="big")
                wv_sb = big.tile([128, 8, DG], MMT, tag="big")
                # tiny PE-gating constants first on the scalar queue, then the
                # transfers that gate the first projection chains
                ones = const.tile([128, 128], MMT, tag="ones")
                nc.scalar.dma_start(out=ones, in_=onesin[:, :])
                bvr_sb = const.tile([1, DG], MMT, tag="bvr")
                nc.scalar.dma_start(out=bvr_sb, in_=bvr[:, :])
                nc.sync.dma_start(out=wq_sb[:, :, 0:128], in_=wq[:, :, 0:128])
                xtc_dma(nc.scalar, 0, 0, 2)
                g(nc.gpsimd).dma_start(out=wk_sb[:, :, 0:128], in_=wk[:, :, 0:128])
                nc.sync.dma_start(out=wq_sb[:, :, 128:DG], in_=wq[:, :, 128:DG])
                xtc_dma(nc.sync, 0, 2, 4)
                g(nc.gpsimd).dma_start(out=wk_sb[:, :, 128:DG], in_=wk[:, :, 128:DG])
                xtc_dma(nc.scalar, 0, 4, 6)
                xtc_dma(nc.gpsimd, 0, 6, 8)
                # chunk 1 next — it gates the third pre-stream chain Q(0,0,1)
                xtc_dma(nc.sync, 1, 0, 2)
                xtc_dma(nc.scalar, 1, 2, 4)
                xtc_dma(nc.gpsimd, 1, 4, 6)
                xtc_dma(nc.sync, 1, 6, 8)
                g(nc.gpsimd).dma_start(out=wv_sb[:, :, 0:128], in_=wv[:, :, 0:128])
                g(nc.gpsimd).dma_start(out=wv_sb[:, :, 128:DG], in_=wv[:, :, 128:DG])

                # ---- remaining small constants ----
                bq_sb = const.tile([128, 2], F32, tag="bq")
                nc.scalar.dma_start(out=bq_sb, in_=bq[:, :])
                bk_sb = const.tile([128, 2], F32, tag="bk")
                nc.scalar.dma_start(out=bk_sb, in_=bk[:, :])
                mb_sb = const.tile([128, 16], F32, tag="mb")
                nc.scalar.dma_start(out=mb_sb, in_=mb[:, :])
                wo_sb = const.tile([128, 2, D], MMT, tag="wo")
                bv_ps = psum_b.tile([128, DG], F32, tag="bank")
                bv_bc = const.tile([128, DG], F32, tag="bvbc")
                # remaining X chunks, chunk-major so early chains can start
                # before the full X load completes
                xtc_dma(nc.sync, 3, 0, 8)
                xtc = [[xtc_t[c][:, kt, :] for kt in range(8)] for c in range(4)]

                qt_sb = [qk.tile([128, N], MMT, tag=f"qt{m}", name=f"qt{m}") for m in range(2)]
                kt_sb = [qk.tile([128, N], MMT, tag=f"kt{m}", name=f"kt{m}") for m in range(2)]
                # V with ones column appended per head: [128, jt, head, 65]
                v_sb = qk.tile([128, 16, HPG, HD + 1], MMT, tag="v")
                g(nc.gpsimd).dma_start(
                    out=v_sb[:, :, :, HD : HD + 1], in_=onesin[:, 0:64]
                )
                xtc_dma(nc.gpsimd, 2, 0, 8)

                def qk_chain(proj, mt, nt):
                    w_sb, bias_sb, dst = (
                        (wq_sb, bq_sb, qt_sb) if proj == 0 else (wk_sb, bk_sb, kt_sb)
                    )
                    ps = psum_b.tile([128, 512], F32, tag="bank", name="qkps")
                    for kt in range(8):
                        nc.tensor.matmul(
                            ps,
                            w_sb[:, kt, mt * 128 : (mt + 1) * 128],
                            xtc[nt][kt],
                            start=(kt == 0),
                            stop=(kt == 7),
                        )
                    nc.vector.tensor_scalar_add(
                        dst[mt][:, nt * 512 : (nt + 1) * 512],
                        ps,
                        bias_sb[:, mt : mt + 1],
                    )

                qk_half = {}

                def qk_chain_part(proj, mt, nt, part, nparts=4):
                    # one kt-slice of a projection chain; partial sums chain
                    # through SBUF so no PSUM slot is held across units
                    w_sb = wq_sb if proj == 0 else wk_sb
                    bias_sb = bq_sb if proj == 0 else bk_sb
                    dst = qt_sb if proj == 0 else kt_sb
                    kpp = 8 // nparts
                    ps = psum_b.tile([128, 512], F32, tag="bank", name="qkpp")
                    for kt in range(part * kpp, (part + 1) * kpp):
                        nc.tensor.matmul(
                            ps,
                            w_sb[:, kt, mt * 128 : (mt + 1) * 128],
                            xtc[nt][kt],
                            start=(kt == part * kpp),
                            stop=(kt == (part + 1) * kpp - 1),
                        )
                    if part == 0:
                        tmp = big.tile(
                            [128, 512], F32, tag="qhalf", bufs=2, name="qh"
                        )
                        nc.vector.tensor_scalar_add(
                            tmp, ps, bias_sb[:, mt : mt + 1]
                        )
                        qk_half[(proj, mt, nt)] = tmp
                    elif part < nparts - 1:
                        tmp = qk_half[(proj, mt, nt)]
                        nc.vector.tensor_tensor(
                            out=tmp, in0=ps, in1=tmp, op=mybir.AluOpType.add
                        )
                    else:
                        nc.vector.tensor_tensor(
                            out=dst[mt][:, nt * 512 : (nt + 1) * 512],
                            in0=ps,
                            in1=qk_half.pop((proj, mt, nt)),
                            op=mybir.AluOpType.add,
                        )

                def v_chain(mt):
                    ps = psum_b.tile([128, DG], F32, tag="bank", name="vps")
                    for kt in range(8):
                        nc.tensor.matmul(
                            ps,
                            xtc[mt // 4][kt][
                                :, (mt % 4) * 128 : (mt % 4 + 1) * 128
                            ],
                            wv_sb[:, kt, :],
                            start=(kt == 0),
                            stop=(kt == 7),
                        )
                    nc.vector.tensor_tensor(
                        out=v_sb[:, mt, :, 0:HD],
                        in0=ps.rearrange("p (h d) -> p h d", h=HPG),
                        in1=bv_bc.rearrange("p (h d) -> p h d", h=HPG),
                        op=mybir.AluOpType.add,
                    )

                # chains needed before the unit stream starts: Q queries 0-1023
                # for head pair 0 and K keys 0-511 for head pair 0. Order
                # [Q nt0, K nt0, Q nt1]: the first two need only X chunk 0, so
                # the K chain fills the chunk-1 arrival window; v_chain(0)
                # (chunk 0 only) runs as the iteration-0 insert.
                for fn in (
                    lambda: qk_chain(0, 0, 0),
                    lambda: qk_chain(1, 0, 0),
                    lambda: qk_chain(0, 0, 1),
                ):
                    fn()
                # bv broadcast to all 128 partitions via PE (needed from
                # v_chain(0) onwards; emitted here so a late bvr DMA can not
                # block the projection chains in the in-order PE queue)
                nc.tensor.matmul(
                    bv_ps, ones[0:1, 0:128], bvr_sb[0:1, :],
                    start=True, stop=True,
                )
                nc.vector.tensor_copy(bv_bc, bv_ps)

                # remaining chains, spread through the unit stream (key =
                # iteration index at whose END the chain is emitted; each must
                # precede its first consumer unit). v_chain(m) is pinned at
                # iteration m (consumed by emit_ctx at iteration m+1); qk
                # chains go as late as their deadline allows so the PE has
                # spare work in the ACT-bound midsection.
                inserts = {}
                for i in range(0, 16):
                    inserts.setdefault(i, []).append(lambda m=i: v_chain(m))
                inserts.setdefault(1, []).append(lambda: qk_chain(1, 0, 1))
                inserts.setdefault(4, []).append(lambda: qk_chain(1, 0, 2))
                inserts.setdefault(7, []).append(lambda: qk_chain(1, 0, 3))
                inserts.setdefault(9, []).append(lambda: qk_chain(0, 1, 0))
                inserts.setdefault(11, []).append(lambda: qk_chain(0, 1, 1))
                inserts.setdefault(8, []).append(
                    lambda: nc.sync.dma_start(out=wo_sb, in_=wo[:, :, :])
                )
                inserts.setdefault(13, []).append(lambda: qk_chain(1, 1, 0))
                inserts.setdefault(16, []).append(lambda: qk_chain(1, 1, 1))
                inserts.setdefault(19, []).append(lambda: qk_chain(1, 1, 2))
                inserts.setdefault(22, []).append(lambda: qk_chain(1, 1, 3))
                inserts.setdefault(25, []).append(lambda: qk_chain(0, 0, 2))
                inserts.setdefault(28, []).append(lambda: qk_chain(0, 0, 3))
                inserts.setdefault(36, []).append(lambda: qk_chain(0, 1, 2))
                inserts.setdefault(42, []).append(lambda: qk_chain(0, 1, 3))

                # ---- phase 2: attention, software-pipelined emission ----
                ctxn = [
                    qk.tile([128, N], MMT, tag=f"ctxn{m}", name=f"ctxn{m}")
                    for m in range(2)
                ]

                blocks = [(ih, hp) for ih in range(2) for hp in range(2)]
                units = [
                    (b_idx, ih, hp, jt)
                    for b_idx, (ih, hp) in enumerate(blocks)
                    for jt in range(16)
                ]
                ctx_ps_of = {}
                unit_e = {}

                def emit_s_exp(u):
                    b_idx, ih, hp, jt = u
                    e_sb = [
                        epool.tile([128, 1024], MMT, tag="e", name="esb")
                        for _ in range(2)
                    ]
                    s_ps2 = [
                        psum_b.tile([128, 1024], F32, tag="bank", name="sps")
                        for _ in range(2)
                    ]
                    # it-outer, h2-inner: consecutive matmuls sit in different
                    # PE row groups (tile_position below) and run concurrently.
                    for it in range(2):
                        for h2 in range(2):
                            nc.tensor.matmul(
                                s_ps2[h2][:, it * 512 : (it + 1) * 512],
                                kt_sb[hp][
                                    h2 * 64 : (h2 + 1) * 64,
                                    jt * 128 : (jt + 1) * 128,
                                ],
                                qt_sb[hp][
                                    h2 * 64 : (h2 + 1) * 64,
                                    ih * 1024 + it * 512 : ih * 1024
                                    + (it + 1) * 512,
                                ],
                                start=True,
                                stop=True,
                                tile_position=(
                                    (h2 * 64, 0) if USE_TILEPOS else None
                                ),
                            )
                    for h2 in range(2):
                        nc.scalar.activation(
                            out=e_sb[h2],
                            in_=s_ps2[h2],
                            func=mybir.ActivationFunctionType.Exp,
                            bias=mb_sb[:, jt : jt + 1],
                            scale=0.125,
                        )
                    unit_e[u] = e_sb

                def emit_ctx(u):
                    b_idx, ih, hp, jt = u
                    if b_idx not in ctx_ps_of:
                        ctx_ps_of[b_idx] = [
                            psum_c.tile([HD + 1, 1024], F32, tag="ctx", name="ctxps")
                            for _ in range(2)
                        ]
                    ctx_ps = ctx_ps_of[b_idx]
                    e_sb = unit_e.pop(u)
                    for h2 in range(2):
                        for it in range(2):
                            nc.tensor.matmul(
                                ctx_ps[h2][:, it * 512 : (it + 1) * 512],
                                v_sb[:, jt, 2 * hp + h2, :],
                                e_sb[h2][:, it * 512 : (it + 1) * 512],
                                start=(jt == 0),
                                stop=(jt == 15),
                                skip_group_check=True,
                            )

                def emit_norm(b_idx, tail=False):
                    ih, hp = blocks[b_idx]
                    ctx_ps = ctx_ps_of[b_idx]
                    # recips first (both h2) so the broadcast/copy chain that
                    # recycles the score-PSUM slots starts as early as possible
                    r = {}
                    for h2 in (1, 0):
                        r_sb = rpool.tile([65, 1024], MMT, tag="r", name="rsb")
                        nc.vector.reciprocal(
                            out=r_sb[64:65, :], in_=ctx_ps[h2][64:65, :]
                        )
                        r[h2] = r_sb
                    for h2 in (1, 0):
                        rp = psum_b.tile([64, 1024], F32, tag="bank", name="rp")
                        for it in range(2):
                            nc.tensor.matmul(
                                rp[:, it * 512 : (it + 1) * 512],
                                ones[64:65, 0:64],
                                r[h2][64:65, it * 512 : (it + 1) * 512],
                                start=True,
                                stop=True,
                                tile_position=(64, 0),
                            )
                        # ScalarE does the drain copy: it is stalled waiting
                        # on these exact PSUM slots anyway
                        nc.scalar.copy(r[h2][0:64, :], rp)
                    # h2=0 mult first: it frees the ctx-PSUM slot the next
                    # block's first (h2=0) ctx matmul is waiting for
                    nc.vector.tensor_tensor(
                        out=ctxn[hp][0:64, ih * 1024 : (ih + 1) * 1024],
                        in0=ctx_ps[0][0:64, :],
                        in1=r[0][0:64, :],
                        op=mybir.AluOpType.mult,
                    )
                    tmp = big.tile([64, 1024], MMT, tag="big", name="tmp")
                    nc.vector.tensor_tensor(
                        out=tmp,
                        in0=ctx_ps[1][0:64, :],
                        in1=r[1][0:64, :],
                        op=mybir.AluOpType.mult,
                    )
                    # partition shift 0-63 -> 64-127 via SBUF->SBUF DMA
                    nc.sync.dma_start(
                        out=ctxn[hp][64:128, ih * 1024 : (ih + 1) * 1024],
                        in_=tmp,
                    )

                def out_dma(ih, mo, ob, nsplit=2, engs=None):
                    # pieces on separate triggers so the transfers ride
                    # different DMA rings and no single queue serializes
                    engs = engs or [nc.sync, nc.gpsimd]
                    w = 1024 // nsplit
                    for s in range(nsplit):
                        engs[s].dma_start(
                            out=outp[
                                mo * 128 : (mo + 1) * 128,
                                ih * 1024 + s * w : ih * 1024 + (s + 1) * w,
                            ],
                            in_=ob[:, s * w : (s + 1) * w],
                        )

                part_sb = {}

                def emit_outproj(ih, mo_list=None, copy_eng=None):
                    for mo in (range(8) if mo_list is None else mo_list):
                        ps = psum_b.tile([128, 1024], F32, tag="bank", name="ops")
                        for nt2 in range(2):
                            nt = 2 * ih + nt2
                            for kt in range(2):
                                nc.tensor.matmul(
                                    ps[:, nt2 * 512 : (nt2 + 1) * 512],
                                    wo_sb[:, kt, mo * 128 : (mo + 1) * 128],
                                    ctxn[kt][:, nt * 512 : (nt + 1) * 512],
                                    start=(kt == 0),
                                    stop=(kt == 1),
                                )
                        ob = big.tile([128, 1024], F32, tag="big", name="ob")
                        (copy_eng or nc.vector.tensor_copy)(ob, ps)
                        if ih == 1 and mo >= 6:
                            out_dma(
                                ih, mo, ob, 4,
                                [nc.sync, nc.scalar, nc.gpsimd, nc.sync],
                            )
                        else:
                            out_dma(ih, mo, ob)

                def emit_outproj_kt0(ih, mo):
                    # first half of the Wo contraction, run while the unit
                    # stream still occupies ACT; result parked in SBUF
                    ps = psum_b.tile([128, 1024], F32, tag="bank", name="opsa")
                    for nt2 in range(2):
                        nt = 2 * ih + nt2
                        nc.tensor.matmul(
                            ps[:, nt2 * 512 : (nt2 + 1) * 512],
                            wo_sb[:, 0, mo * 128 : (mo + 1) * 128],
                            ctxn[0][:, nt * 512 : (nt + 1) * 512],
                            start=True,
                            stop=True,
                        )
                    pt = big.tile(
                        [128, 1024], F32, tag="part", bufs=4, name="pt"
                    )
                    nc.vector.tensor_copy(pt, ps)
                    part_sb[(ih, mo)] = pt

                def emit_outproj_kt1(ih, mo):
                    ps = psum_b.tile([128, 1024], F32, tag="bank", name="opsb")
                    for nt2 in range(2):
                        nt = 2 * ih + nt2
                        nc.tensor.matmul(
                            ps[:, nt2 * 512 : (nt2 + 1) * 512],
                            wo_sb[:, 1, mo * 128 : (mo + 1) * 128],
                            ctxn[1][:, nt * 512 : (nt + 1) * 512],
                            start=True,
                            stop=True,
                        )
                    ob = big.tile([128, 1024], F32, tag="big", name="ob")
                    nc.vector.tensor_tensor(
                        out=ob,
                        in0=ps,
                        in1=part_sb.pop((ih, mo)),
                        op=mybir.AluOpType.add,
                    )
                    if ih == 1 and mo >= 6:
                        out_dma(
                            ih, mo, ob, 4,
                            [nc.sync, nc.scalar, nc.gpsimd, nc.sync],
                        )
                    else:
                        out_dma(ih, mo, ob)

                extras = {}
                for b_idx, (ih, hp) in enumerate(blocks):
                    last = 16 * (b_idx + 1) - 1
                    extras.setdefault(last + 1, []).append(
                        lambda b=b_idx, t=(b_idx == 3): emit_norm(b, tail=t)
                    )
                # outproj for ih=0 spread through units 47..61; for ih=1 the
                # even-mo kt=0 partials also run in-stream (units 49..63, after
                # norm(2) publishes ctxn[0] ih=1), so the tail only runs the
                # kt=1 halves. Odd-mo tail chains use the (idle) ScalarE for
                # the PSUM drain copy so DVE and ACT alternate.
                for j, mo in enumerate(range(8)):
                    extras.setdefault(47 + 2 * j, []).append(
                        lambda m=mo: emit_outproj(0, [m])
                    )
                for j, mo in enumerate((0, 2, 4, 6)):
                    extras.setdefault(50 + 3 * j, []).append(
                        lambda m=mo: emit_outproj_kt0(1, m)
                    )
                for j, mo in enumerate(range(8)):
                    if mo % 2 == 0:
                        extras.setdefault(64 + 3 + j, []).append(
                            lambda m=mo: emit_outproj_kt1(1, m)
                        )
                    else:
                        extras.setdefault(64 + 3 + j, []).append(
                            lambda m=mo: emit_outproj(1, [m], nc.scalar.copy)
                        )

                trailing = []
                for i, u in enumerate(units):
                    emit_s_exp(u)
                    if i > 0:
                        emit_ctx(units[i - 1])
                    for fn in inserts.get(i, []):
                        fn()
                    for fn in extras.get(i, []):
                        if i == len(units) - 1:
                            trailing.append(fn)
                        else:
                            fn()
                emit_ctx(units[-1])
                for i in sorted(extras):
                    if i >= len(units):
                        trailing.extend(extras[i])
                for fn in trailing:
                    fn()

    nc.finalize()
    return nc


_NC_CACHE = None


def _get_program():
    global _NC_CACHE
    if _NC_CACHE is None:
        _NC_CACHE = build_program()
    return _NC_CACHE


def make_in_maps(X, mask, Wq, bq, Wk, bk, Wv, bv, Wo, bo):
    import ml_dtypes

    BF16 = ml_dtypes.bfloat16
    X = np.asarray(X, dtype=np.float32)
    mask = np.asarray(mask, dtype=np.float32)
    in_maps = []
    xts = [np.ascontiguousarray(X[b].T).astype(BF16) for b in range(B)]
    mbs = [
        np.ascontiguousarray((-1e6 * (1.0 - mask[b])).reshape(16, 128).T)
        for b in range(B)
    ]
    ones_bf = np.ones((128, 128), dtype=BF16)
    for c in range(8):
        b, g = c // HG, c % HG
        sl = slice(g * DG, (g + 1) * DG)
        wq_s = np.ascontiguousarray(
            np.asarray(Wq[:, sl]).reshape(8, 128, DG).transpose(1, 0, 2)
        )
        wk_s = np.ascontiguousarray(
            np.asarray(Wk[:, sl]).reshape(8, 128, DG).transpose(1, 0, 2)
        )
        wv_s = np.ascontiguousarray(
            np.asarray(Wv[:, sl]).reshape(8, 128, DG).transpose(1, 0, 2)
        )
        bq_s = np.ascontiguousarray(np.asarray(bq[sl]).reshape(2, 128).T)
        bk_s = np.ascontiguousarray(np.asarray(bk[sl]).reshape(2, 128).T)
        bv_s = np.ascontiguousarray(np.asarray(bv[sl]).reshape(1, DG))
        # Wo rows for this group, pair-packed: [64*h2+p, kt, o] = Wo[g*256+(2kt+h2)*64+p, o]
        wo_s = np.ascontiguousarray(
            np.asarray(Wo[sl, :]).reshape(2, 2, 64, D).transpose(1, 2, 0, 3)
            .reshape(128, 2, D)
        )
        in_maps.append(
            {
                "xt": xts[b],
                "onesin": ones_bf,
                "wq": wq_s.astype(BF16),
                "wk": wk_s.astype(BF16),
                "wv": wv_s.astype(BF16),
                "bq": bq_s.astype(np.float32),
                "bk": bk_s.astype(np.float32),
                "bvr": bv_s.astype(BF16),
                "wo": wo_s.astype(BF16),
                "mb": mbs[b].astype(np.float32),
            }
        )
    return in_maps


def gather_output(results, bo):
    out = np.zeros((B, N, D), dtype=np.float32)
    for c in range(8):
        out[c // HG] += results[c]["outp"].T
    out += np.asarray(bo, dtype=np.float32)
    return out


def kernel(**inputs):
    from concourse import bass_utils

    nc = _get_program()
    in_maps = make_in_maps(**inputs)
    res = bass_utils.run_bass_kernel_spmd(nc, in_maps, core_ids=list(range(8)))
    return gather_output(res.results, inputs["bo"])
